# revision 29
# baseline (speedup 1.0000x reference)
"""Trainium2 Bass kernel for the ActorCriticSNN LIF network (DSQN drone).

Strategy (data-parallel over batch, 16 elements per core, 8 cores):
  The network is feedforward ACROSS layers (each layer's recurrence is internal),
  so it decomposes into: big time-batched GEMMs between layers (TensorE) +
  per-layer sequential LIF scans (VectorE) + a linear output accumulator
  (tensor_tensor_scan).

  Normalized coordinates u = (mem - thr)/thr make the LIF step:
      u_t = beta*u_{t-1} + c_t - s_{t-1},   s_t = (u_t > 0)
  With the pre-decay state ubar = beta*u, each step is 3 VectorE ops:
      i2: a = ubar - cms          (a == u_t)
      i1: cms' = (a > 0) - c_{t+1}   [scalar_tensor_tensor, fused is_gt]
      i3: ubar = a * beta_tile
  Spikes for the GEMMs are extracted on ScalarE as Sign(a) in {-1,+1} (bf16,
  exact); the +-1 encoding is folded into host-precomputed weights/biases.
  GEMM weights are split into bf16 hi+lo pairs -> fp32-class accuracy at
  bf16 matmul speed. The two layer scans are interleaved tick-by-tick
  (layer 2 delayed by D ticks) so independent DVE ops pipeline and hide
  the DVE pipe-drain.
"""
import sys
import numpy as np

sys.path.insert(0, '/opt/trn_rl_repo')

import concourse.bass as bass  # noqa: E402
import concourse.tile as tile  # noqa: E402
from concourse import bacc, mybir  # noqa: E402
from concourse.bass_utils import run_bass_kernel_spmd  # noqa: E402

import ml_dtypes  # noqa: E402

# Problem constants (hardcoded per spec)
B, T, NIN, H, NACT = 128, 256, 16, 512, 4
N_CORES = 8
BL = B // N_CORES          # 16 batch per core
TB = 8                     # steps per pipeline block
D = 16                     # scan2 delay (ticks) behind scan1; multiple of TB
LAST_TICK = T - 1 + D      # 287
SC = 4 * BL                # step-column-block for layer arrays (64)
NQ = 8                     # host-precomputed c1 DMA chunks
QS = 33                    # t-steps per c1 chunk (8*33 >= 257)

BF16 = ml_dtypes.bfloat16

_cache = {}


def _bf16(x):
    return np.asarray(x, np.float32).astype(BF16)


def _bf16_split(x):
    hi = _bf16(x)
    lo = _bf16(np.asarray(x, np.float32) - hi.astype(np.float32))
    return hi, lo


def _build_program():
    """Build the per-core Bass program (same NEFF on all 8 cores)."""
    fp32 = mybir.dt.float32
    bf16 = mybir.dt.bfloat16
    Sign = mybir.ActivationFunctionType.Sign
    Ident = mybir.ActivationFunctionType.Identity
    Op = mybir.AluOpType

    nc = bacc.Bacc("TRN2", target_bir_lowering=False, debug=False,
                   num_devices=N_CORES)

    # ---- DRAM parameters ----
    # c1 = W1n @ x + b1n is input-only -> host-precomputed (exact fp32),
    # DMA'd in NQ chunks so the scan can start as soon as chunk 0 lands.
    c1q_e = [nc.dram_tensor(f"c1q{i}", [128, QS * SC], fp32,
                            kind="ExternalInput").ap() for i in range(NQ)]
    w2hi_e = nc.dram_tensor("w2hi", [128, 16 * 128], bf16, kind="ExternalInput").ap()
    w2lo_e = nc.dram_tensor("w2lo", [128, 16 * 128], bf16, kind="ExternalInput").ap()
    wahi_e = nc.dram_tensor("wahi", [128, 16], bf16, kind="ExternalInput").ap()
    walo_e = nc.dram_tensor("walo", [128, 16], bf16, kind="ExternalInput").ap()
    b2n_e = nc.dram_tensor("b2n", [128, 4], fp32, kind="ExternalInput").ap()
    ban_e = nc.dram_tensor("ban", [NACT, 1], fp32, kind="ExternalInput").ap()
    bt1_e = nc.dram_tensor("bt1", [128, SC], fp32, kind="ExternalInput").ap()
    nbt1_e = nc.dram_tensor("nbt1", [128, SC], fp32, kind="ExternalInput").ap()
    bt2_e = nc.dram_tensor("bt2", [128, SC], fp32, kind="ExternalInput").ap()
    nbt2_e = nc.dram_tensor("nbt2", [128, SC], fp32, kind="ExternalInput").ap()
    out_e = nc.dram_tensor("out", [4 * BL, T], fp32, kind="ExternalOutput").ap()

    NB2 = T // TB                  # c2 blocks (t = 0..255)
    NB3 = T // TB                  # act blocks

    with tile.TileContext(nc) as tc:
        import contextlib
        with contextlib.ExitStack() as ctx:
            consts = ctx.enter_context(tc.tile_pool(name="consts", bufs=1))
            c2p = ctx.enter_context(tc.tile_pool(name="c2p", bufs=4))
            s12p = ctx.enter_context(tc.tile_pool(name="s12p", bufs=4))
            ps2p = ctx.enter_context(tc.tile_pool(name="ps2p", bufs=5, space="PSUM"))
            ps3p = ctx.enter_context(tc.tile_pool(name="ps3p", bufs=3, space="PSUM"))

            # ---- load constants ----
            c1q = [consts.tile([128, QS * SC], fp32, name=f"c1q{i}")
                   for i in range(NQ)]
            for i in range(NQ):
                nc.sync.dma_start(out=c1q[i], in_=c1q_e[i])
            w2hi = consts.tile([128, 16 * 128], bf16)
            w2lo = consts.tile([128, 16 * 128], bf16)
            wahi = consts.tile([128, 16], bf16)
            walo = consts.tile([128, 16], bf16)
            b2n = consts.tile([128, 4], fp32)
            ban = consts.tile([NACT, 1], fp32)
            bt1 = consts.tile([128, SC], fp32)
            nbt1 = consts.tile([128, SC], fp32)
            bt2 = consts.tile([128, SC], fp32)
            nbt2 = consts.tile([128, SC], fp32)
            for dst, src in [(w2hi, w2hi_e), (w2lo, w2lo_e),
                             (wahi, wahi_e), (walo, walo_e), (b2n, b2n_e), (ban, ban_e), (bt1, bt1_e),
                             (nbt1, nbt1_e), (bt2, bt2_e), (nbt2, nbt2_e)]:
                nc.sync.dma_start(out=dst, in_=src)

            # scan state tiles
            u1 = consts.tile([128, SC], fp32)
            cms1 = consts.tile([128, SC], fp32)
            u2 = consts.tile([128, SC], fp32)
            cms2 = consts.tile([128, SC], fp32)
            act_arr = consts.tile([NACT, BL * T], fp32)     # col = b*T + t
            act64 = consts.tile([4 * BL, T], fp32)          # part = a*BL + b
            decay = consts.tile([4 * BL, T], fp32)
            out_sb = consts.tile([4 * BL, T], fp32)

            nc.vector.tensor_copy(u1, nbt1)
            nc.vector.tensor_copy(u2, nbt2)
            nc.vector.memset(decay, 0.95)
            nc.vector.memset(decay[:, 0:1], 0.0)

            # trigger the ACT table load early, overlapped with input DMAs
            actwarm = consts.tile([4, 1], fp32)
            nc.vector.memset(actwarm, 0.0)
            nc.scalar.activation(out=actwarm, in_=actwarm, func=Sign)

            NA = 6
            a12 = []
            for i in range(NA):
                t_ = consts.tile([128, 2 * SC], fp32, name=f"a12_{i}")
                nc.vector.memset(t_, 0.0)
                a12.append(t_)

            c2_blocks, s12_blocks = {}, {}

            def c1_slice(nt, lo, hi):
                return c1q[nt // QS][:, (nt % QS) * SC + lo:(nt % QS) * SC + hi]

            def c2_slice(nt, lo, hi):
                return c2_blocks[nt // TB][:, (nt % TB) * SC + lo:(nt % TB) * SC + hi]

            def g2_block(k):
                """c2 block k: t in [TB*k, TB*k+TB); needs S12 block k."""
                t0 = TB * k
                ncols = TB * BL
                sblk = s12_blocks[k]
                cblk = c2p.tile([128, TB * SC], fp32, name=f"c2b{k}", tag="c2roll")
                c2_blocks[k] = cblk
                srear = sblk.rearrange("p (t c) -> p t c", c=2 * SC)
                for m in range(4):
                    ps = ps2p.tile([128, TB * BL], fp32, name=f"ps2_{k}_{m}", tag="ps2")
                    mm = 0
                    for j in range(4):
                        rhs = srear[:, :, j * BL:(j + 1) * BL]
                        for w in (w2hi, w2lo):
                            nc.tensor.matmul(
                                ps,
                                w[:, (j * 4 + m) * 128:(j * 4 + m + 1) * 128],
                                rhs,
                                start=(mm == 0), stop=(mm == 7))
                            mm += 1
                    nc.scalar.activation(
                        out=cblk.rearrange("p (t c) -> p t c", c=SC)[:, :, m * BL:(m + 1) * BL],
                        in_=ps.rearrange("p (t b) -> p t b", b=BL),
                        func=Ident, bias=b2n[:, m:m + 1], scale=1.0)

            def g3_block(k):
                """act block k: t in [TB*k, TB*k+TB); spm2_t sits in S12 block k+2."""
                t0 = TB * k
                ps = ps3p.tile([NACT, TB * BL], fp32, name=f"ps3_{k}", tag="ps3")
                sblk = s12_blocks[k + D // TB]
                srear = sblk.rearrange("p (t c) -> p t c", c=2 * SC)
                mm = 0
                for j in range(4):
                    rhs = srear[:, :, SC + j * BL: SC + (j + 1) * BL]
                    for w in (wahi, walo):
                        nc.tensor.matmul(
                            ps,
                            w[:, j * 4:(j + 1) * 4],
                            rhs,
                            start=(mm == 0), stop=(mm == 7))
                        mm += 1
                # epilogue -> act_arr (b-major): col = b*T + t0 + i
                nc.scalar.activation(
                    out=act_arr.rearrange("p (b t) -> p b t", t=T)[:, :, t0:t0 + TB],
                    in_=ps.rearrange("p (t b) -> p b t", b=BL),
                    func=Ident, bias=ban, scale=1.0)

            # ---- prologue: scan1 init from host-provided c1 ----
            nc.vector.tensor_scalar_mul(cms1, c1_slice(0, 0, SC), -1.0)

            # ---- main tick loop ----
            for tk in range(LAST_TICK + 1):
                if tk % TB == 0:
                    kb = tk // TB
                    sblk = s12p.tile([128, TB * 2 * SC], bf16,
                                     name=f"s12b{kb}", tag="s12roll")
                    s12_blocks[kb] = sblk
                    if 0 <= kb - 1 <= NB2 - 1:
                        g2_block(kb - 1)
                    if 0 <= kb - 3 <= NB3 - 2:
                        g3_block(kb - 3)
                a = a12[tk % NA]
                sig = tk - D
                if sig == -1:
                    nc.vector.tensor_scalar_mul(cms2, c2_slice(0, 0, SC), -1.0)
                # active (layer, col-range) units; single layer splits into two
                # independent half-feature chains so DVE ops still pipeline
                l1_on = tk < T
                l2_on = 0 <= sig < T
                units = []
                if l1_on:
                    rngs = [(0, SC)] if l2_on else [(0, SC // 2), (SC // 2, SC)]
                    for lo, hi in rngs:
                        units.append((u1, cms1, bt1, c1_slice, tk + 1, lo, hi, 0))
                if l2_on:
                    rngs = [(0, SC)] if l1_on else [(0, SC // 2), (SC // 2, SC)]
                    for lo, hi in rngs:
                        units.append((u2, cms2, bt2, c2_slice, sig + 1,
                                      lo, hi, SC))
                # i2
                for u, cms, bt, cfn, nt, lo, hi, off in units:
                    nc.vector.tensor_tensor(
                        out=a[:, off + lo:off + hi], in0=u[:, lo:hi],
                        in1=cms[:, lo:hi], op=Op.subtract)
                # i1 (skip layer-2's last: its output is never consumed)
                for u, cms, bt, cfn, nt, lo, hi, off in units:
                    if off == SC and nt >= T:
                        continue
                    nc.vector.scalar_tensor_tensor(
                        out=cms[:, lo:hi], in0=a[:, off + lo:off + hi], scalar=0.0,
                        in1=cfn(nt, lo, hi),
                        op0=Op.is_gt, op1=Op.subtract)
                # i3
                for u, cms, bt, cfn, nt, lo, hi, off in units:
                    nc.vector.tensor_tensor(
                        out=u[:, lo:hi], in0=a[:, off + lo:off + hi],
                        in1=bt[:, lo:hi], op=Op.mult)
                # i4 (ScalarE): spikes as Sign(a) in {-1, +1}
                sblk = s12_blocks[tk // TB]
                base = (tk % TB) * 2 * SC
                nc.scalar.activation(
                    out=sblk[:, base:base + 2 * SC],
                    in_=a, func=Sign)

            # ---- tail: last act block, transpose, output scan, DMA out ----
            # first half (act blocks 0..NB3/2-1 are long done): transpose + scan
            TH = T // 2
            nc.sync.dma_start(
                out=act64[:, :TH],
                in_=act_arr.rearrange("p (b t) -> p b t", t=T)[:, :, :TH])
            nc.vector.tensor_tensor_scan(
                out=out_sb[:, :TH], data0=decay[:, :TH], data1=act64[:, :TH],
                initial=0.0, op0=Op.mult, op1=Op.add)
            nc.sync.dma_start(out=out_e[:, :TH], in_=out_sb[:, :TH])
            g3_block(NB3 - 1)
            nc.sync.dma_start(
                out=act64[:, TH:],
                in_=act_arr.rearrange("p (b t) -> p b t", t=T)[:, :, TH:])
            nc.vector.tensor_tensor_scan(
                out=out_sb[:, TH:], data0=decay[:, TH:], data1=act64[:, TH:],
                initial=out_sb[:, TH - 1:TH], op0=Op.mult, op1=Op.add)
            nc.sync.dma_start(out=out_e[:, TH:], in_=out_sb[:, TH:])

    nc.compile()
    return nc


def _prep_inputs(inputs):
    """Host-side prep: normalized split-precision weights + per-core shards."""
    x = np.asarray(inputs["batch"], np.float32)        # [B, T, NIN]
    W1 = np.asarray(inputs["W1"], np.float32); b1 = np.asarray(inputs["b1"], np.float32)
    W2 = np.asarray(inputs["W2"], np.float32); b2 = np.asarray(inputs["b2"], np.float32)
    Wa = np.asarray(inputs["Wa"], np.float32); ba = np.asarray(inputs["ba"], np.float32)
    beta1 = np.clip(np.asarray(inputs["beta1"], np.float32), 0, 1)
    thr1 = np.asarray(inputs["thr1"], np.float32)
    beta2 = np.clip(np.asarray(inputs["beta2"], np.float32), 0, 1)
    thr2 = np.asarray(inputs["thr2"], np.float32)
    mn = float(np.float32(inputs["inp_min"])); mx = float(np.float32(inputs["inp_max"]))
    R = mx - mn

    W1n = (W1 / R) / thr1[:, None]
    b1eff = b1 - (mn / R) * W1.sum(1)
    b1n = b1eff / thr1 + beta1 - 1.0

    W2n = W2 / thr2[:, None]
    b2n = b2 / thr2 + beta2 - 1.0
    W2e = W2n / 2
    b2tot = b2n + W2n.sum(1) / 2
    W2hi, W2lo = _bf16_split(W2e)

    Wae = Wa / 2
    batot = ba + Wa.sum(1) / 2
    Wahi, Walo = _bf16_split(Wae)

    def chunked_w2(w):  # [512,512] -> W2eT chunk layout: col (j*4+m)*128 + mc
        wt = np.asarray(w).T
        outw = np.zeros((128, 16 * 128), w.dtype)
        for j in range(4):
            for m in range(4):
                outw[:, (j * 4 + m) * 128:(j * 4 + m + 1) * 128] = \
                    wt[j * 128:(j + 1) * 128, m * 128:(m + 1) * 128]
        return outw

    def chunked_wa(w):  # [4,512] -> WaeT chunks: col j*4 + a
        wt = np.asarray(w).T
        outw = np.zeros((128, 16), w.dtype)
        for j in range(4):
            outw[:, j * 4:(j + 1) * 4] = wt[j * 128:(j + 1) * 128, :]
        return outw

    def beta_tile(beta):
        return np.ascontiguousarray(
            np.repeat(beta.reshape(4, 128).T[:, :, None], BL, 2).reshape(128, SC))

    common = {
        "w2hi": np.ascontiguousarray(chunked_w2(W2hi)),
        "w2lo": np.ascontiguousarray(chunked_w2(W2lo)),
        "wahi": np.ascontiguousarray(chunked_wa(Wahi)),
        "walo": np.ascontiguousarray(chunked_wa(Walo)),
        "b2n": np.ascontiguousarray(b2tot.reshape(4, 128).T),
        "ban": np.ascontiguousarray(batot.reshape(NACT, 1)),
        "bt1": beta_tile(beta1),
        "bt2": beta_tile(beta2),
    }
    common["nbt1"] = np.ascontiguousarray(-common["bt1"])
    common["nbt2"] = np.ascontiguousarray(-common["bt2"])

    # per-core host-precomputed c1 = W1n @ x_t + b1n (exact fp32),
    # laid out [128, t*SC + j*BL + b] and split into NQ chunks of QS steps
    xt = x.transpose(1, 0, 2)  # [T, B, NIN]
    in_maps = []
    for c in range(N_CORES):
        xs = xt[:, c * BL:(c + 1) * BL, :]                    # [T, BL, NIN]
        c1 = np.einsum('hk,tbk->thb', W1n, xs).astype(np.float32) \
            + b1n[None, :, None]                              # [T, 512, BL]
        c1 = np.concatenate(
            [c1, np.broadcast_to(b1n[None, :, None], (1, H, BL))], 0)  # t=256
        c1c = np.ascontiguousarray(
            c1.reshape(T + 1, 4, 128, BL).transpose(2, 0, 1, 3)
            .reshape(128, (T + 1) * SC))
        pad = np.zeros((128, NQ * QS * SC - (T + 1) * SC), np.float32)
        c1full = np.concatenate([c1c, pad], 1)
        m = dict(common)
        for i in range(NQ):
            m[f"c1q{i}"] = np.ascontiguousarray(
                c1full[:, i * QS * SC:(i + 1) * QS * SC])
        in_maps.append(m)
    return in_maps


def _get_nc():
    if "nc" not in _cache:
        _cache["nc"] = _build_program()
    return _cache["nc"]


def _run(inputs, trace=False, trace_kwargs=None):
    nc = _get_nc()
    in_maps = _prep_inputs(inputs)
    res = run_bass_kernel_spmd(nc, in_maps, core_ids=list(range(N_CORES)),
                               trace=trace, **(trace_kwargs or {}))
    outs = []
    for c in range(N_CORES):
        o = np.asarray(res.results[c]["out"], np.float32)  # [(a,b), t]
        outs.append(o.reshape(NACT, BL, T).transpose(2, 1, 0))  # [T, BL, 4]
    full = np.concatenate(outs, axis=1)          # [T, B, 4]
    return full.reshape(1, T, B * NACT).astype(np.float32), res


def kernel(**inputs) -> np.ndarray:
    out, _ = _run(inputs, trace=False)
    return out


# revision 31
# speedup vs baseline: 1.0427x; 1.0427x over previous
"""Trainium2 Bass kernel for the ActorCriticSNN LIF network (DSQN drone).

Strategy (data-parallel over batch, 16 elements per core, 8 cores):
  The network is feedforward ACROSS layers (each layer's recurrence is internal),
  so it decomposes into: big time-batched GEMMs between layers (TensorE) +
  per-layer sequential LIF scans (VectorE) + a linear output accumulator
  (tensor_tensor_scan).

  Normalized coordinates u = (mem - thr)/thr make the LIF step:
      u_t = beta*u_{t-1} + c_t - s_{t-1},   s_t = (u_t > 0)
  With the pre-decay state ubar = beta*u, each step is 3 VectorE ops:
      i2: a = ubar - cms          (a == u_t)
      i1: cms' = (a > 0) - c_{t+1}   [scalar_tensor_tensor, fused is_gt]
      i3: ubar = a * beta_tile
  Spikes for the GEMMs are extracted on ScalarE as Sign(a) in {-1,+1} (bf16,
  exact); the +-1 encoding is folded into host-precomputed weights/biases.
  GEMM weights are split into bf16 hi+lo pairs -> fp32-class accuracy at
  bf16 matmul speed. The two layer scans are interleaved tick-by-tick
  (layer 2 delayed by D ticks) so independent DVE ops pipeline and hide
  the DVE pipe-drain.
"""
import sys
import numpy as np

sys.path.insert(0, '/opt/trn_rl_repo')

import concourse.bass as bass  # noqa: E402
import concourse.tile as tile  # noqa: E402
from concourse import bacc, mybir  # noqa: E402
from concourse.bass_utils import run_bass_kernel_spmd  # noqa: E402

import ml_dtypes  # noqa: E402

# Problem constants (hardcoded per spec)
B, T, NIN, H, NACT = 128, 256, 16, 512, 4
N_CORES = 8
BL = B // N_CORES          # 16 batch per core
TB = 8                     # steps per pipeline block
D = 16                     # scan2 delay (ticks) behind scan1; multiple of TB
LAST_TICK = T - 1 + D      # 287
SC = 4 * BL                # step-column-block for layer arrays (64)
NQ = 8                     # host-precomputed c1 DMA chunks
QS = 33                    # t-steps per c1 chunk (8*33 >= 257)

BF16 = ml_dtypes.bfloat16

_cache = {}


def _bf16(x):
    return np.asarray(x, np.float32).astype(BF16)


def _bf16_split(x):
    hi = _bf16(x)
    lo = _bf16(np.asarray(x, np.float32) - hi.astype(np.float32))
    return hi, lo


def _build_program():
    """Build the per-core Bass program (same NEFF on all 8 cores)."""
    fp32 = mybir.dt.float32
    bf16 = mybir.dt.bfloat16
    Sign = mybir.ActivationFunctionType.Sign
    Ident = mybir.ActivationFunctionType.Identity
    Op = mybir.AluOpType

    nc = bacc.Bacc("TRN2", target_bir_lowering=False, debug=False,
                   num_devices=N_CORES)

    # ---- DRAM parameters ----
    # c1 = W1n @ x + b1n is input-only -> host-precomputed (exact fp32),
    # DMA'd in NQ chunks so the scan can start as soon as chunk 0 lands.
    c1q_e = [nc.dram_tensor(f"c1q{i}", [128, QS * SC], fp32,
                            kind="ExternalInput").ap() for i in range(NQ)]
    w2hi_e = nc.dram_tensor("w2hi", [128, 16 * 128], bf16, kind="ExternalInput").ap()
    w2lo_e = nc.dram_tensor("w2lo", [128, 16 * 128], bf16, kind="ExternalInput").ap()
    wahi_e = nc.dram_tensor("wahi", [128, 16], bf16, kind="ExternalInput").ap()
    walo_e = nc.dram_tensor("walo", [128, 16], bf16, kind="ExternalInput").ap()
    b2n_e = nc.dram_tensor("b2n", [128, 4], fp32, kind="ExternalInput").ap()
    ban_e = nc.dram_tensor("ban", [NACT, 1], fp32, kind="ExternalInput").ap()
    bt1_e = nc.dram_tensor("bt1", [128, SC], fp32, kind="ExternalInput").ap()
    nbt1_e = nc.dram_tensor("nbt1", [128, SC], fp32, kind="ExternalInput").ap()
    bt2_e = nc.dram_tensor("bt2", [128, SC], fp32, kind="ExternalInput").ap()
    nbt2_e = nc.dram_tensor("nbt2", [128, SC], fp32, kind="ExternalInput").ap()
    out_e = nc.dram_tensor("out", [4 * BL, T], fp32, kind="ExternalOutput").ap()

    NB2 = T // TB                  # c2 blocks (t = 0..255)
    NB3 = T // TB                  # act blocks

    with tile.TileContext(nc) as tc:
        import contextlib
        with contextlib.ExitStack() as ctx:
            consts = ctx.enter_context(tc.tile_pool(name="consts", bufs=1))
            c2p = ctx.enter_context(tc.tile_pool(name="c2p", bufs=4))
            s12p = ctx.enter_context(tc.tile_pool(name="s12p", bufs=4))
            ps2p = ctx.enter_context(tc.tile_pool(name="ps2p", bufs=5, space="PSUM"))
            ps3p = ctx.enter_context(tc.tile_pool(name="ps3p", bufs=3, space="PSUM"))

            # ---- load constants ----
            c1q = [consts.tile([128, QS * SC], fp32, name=f"c1q{i}")
                   for i in range(NQ)]
            w2hi = consts.tile([128, 16 * 128], bf16)
            w2lo = consts.tile([128, 16 * 128], bf16)
            wahi = consts.tile([128, 16], bf16)
            walo = consts.tile([128, 16], bf16)
            b2n = consts.tile([128, 4], fp32)
            ban = consts.tile([NACT, 1], fp32)
            bt1 = consts.tile([128, SC], fp32)
            nbt1 = consts.tile([128, SC], fp32)
            bt2 = consts.tile([128, SC], fp32)
            nbt2 = consts.tile([128, SC], fp32)
            for dst, src in [(bt1, bt1_e), (nbt1, nbt1_e), (bt2, bt2_e),
                             (nbt2, nbt2_e), (b2n, b2n_e), (ban, ban_e),
                             (w2hi, w2hi_e), (w2lo, w2lo_e),
                             (wahi, wahi_e), (walo, walo_e)]:
                nc.sync.dma_start(out=dst, in_=src)
            # big c1 chunks AFTER the small constants so the scan init
            # and first ticks aren't queued behind 8 MB of DMA
            for i in range(NQ):
                nc.sync.dma_start(out=c1q[i], in_=c1q_e[i])

            # scan state tiles
            u1 = consts.tile([128, SC], fp32)
            cms1 = consts.tile([128, SC], fp32)
            u2 = consts.tile([128, SC], fp32)
            cms2 = consts.tile([128, SC], fp32)
            act_arr = consts.tile([NACT, BL * T], fp32)     # col = b*T + t
            act64 = consts.tile([4 * BL, T], fp32)          # part = a*BL + b
            decay = consts.tile([4 * BL, T], fp32)
            out_sb = consts.tile([4 * BL, T], fp32)

            nc.vector.tensor_copy(u1, nbt1)
            nc.vector.tensor_copy(u2, nbt2)
            nc.vector.memset(decay, 0.95)
            nc.vector.memset(decay[:, 0:1], 0.0)

            # trigger the ACT table load early, overlapped with input DMAs
            actwarm = consts.tile([4, 1], fp32)
            nc.vector.memset(actwarm, 0.0)
            nc.scalar.activation(out=actwarm, in_=actwarm, func=Sign)

            NA = 6
            a12 = []
            for i in range(NA):
                t_ = consts.tile([128, 2 * SC], fp32, name=f"a12_{i}")
                nc.vector.memset(t_, 0.0)
                a12.append(t_)

            c2_blocks, s12_blocks = {}, {}

            def c1_slice(nt, lo, hi):
                return c1q[nt // QS][:, (nt % QS) * SC + lo:(nt % QS) * SC + hi]

            def c2_slice(nt, lo, hi):
                return c2_blocks[nt // TB][:, (nt % TB) * SC + lo:(nt % TB) * SC + hi]

            def g2_block(k):
                """c2 block k: t in [TB*k, TB*k+TB); needs S12 block k."""
                t0 = TB * k
                ncols = TB * BL
                sblk = s12_blocks[k]
                cblk = c2p.tile([128, TB * SC], fp32, name=f"c2b{k}", tag="c2roll")
                c2_blocks[k] = cblk
                srear = sblk.rearrange("p (t c) -> p t c", c=2 * SC)
                for m in range(4):
                    ps = ps2p.tile([128, TB * BL], fp32, name=f"ps2_{k}_{m}", tag="ps2")
                    mm = 0
                    for j in range(4):
                        rhs = srear[:, :, j * BL:(j + 1) * BL]
                        for w in (w2hi, w2lo):
                            nc.tensor.matmul(
                                ps,
                                w[:, (j * 4 + m) * 128:(j * 4 + m + 1) * 128],
                                rhs,
                                start=(mm == 0), stop=(mm == 7))
                            mm += 1
                    nc.scalar.activation(
                        out=cblk.rearrange("p (t c) -> p t c", c=SC)[:, :, m * BL:(m + 1) * BL],
                        in_=ps.rearrange("p (t b) -> p t b", b=BL),
                        func=Ident, bias=b2n[:, m:m + 1], scale=1.0)

            def g3_block(k):
                """act block k: t in [TB*k, TB*k+TB); spm2_t sits in S12 block k+2."""
                t0 = TB * k
                ps = ps3p.tile([NACT, TB * BL], fp32, name=f"ps3_{k}", tag="ps3")
                sblk = s12_blocks[k + D // TB]
                srear = sblk.rearrange("p (t c) -> p t c", c=2 * SC)
                mm = 0
                for j in range(4):
                    rhs = srear[:, :, SC + j * BL: SC + (j + 1) * BL]
                    for w in (wahi, walo):
                        nc.tensor.matmul(
                            ps,
                            w[:, j * 4:(j + 1) * 4],
                            rhs,
                            start=(mm == 0), stop=(mm == 7))
                        mm += 1
                # epilogue -> act_arr (b-major): col = b*T + t0 + i
                nc.scalar.activation(
                    out=act_arr.rearrange("p (b t) -> p b t", t=T)[:, :, t0:t0 + TB],
                    in_=ps.rearrange("p (t b) -> p b t", b=BL),
                    func=Ident, bias=ban, scale=1.0)

            # ---- prologue: scan1 init from host-provided c1 ----
            nc.vector.tensor_scalar_mul(cms1, c1_slice(0, 0, SC), -1.0)

            # ---- main tick loop ----
            for tk in range(LAST_TICK + 1):
                if tk % TB == 0:
                    kb = tk // TB
                    sblk = s12p.tile([128, TB * 2 * SC], bf16,
                                     name=f"s12b{kb}", tag="s12roll")
                    s12_blocks[kb] = sblk
                    if 0 <= kb - 1 <= NB2 - 1:
                        g2_block(kb - 1)
                    if 0 <= kb - 3 <= NB3 - 2:
                        g3_block(kb - 3)
                a = a12[tk % NA]
                sig = tk - D
                if sig == -1:
                    nc.vector.tensor_scalar_mul(cms2, c2_slice(0, 0, SC), -1.0)
                # active (layer, col-range) units; single layer splits into two
                # independent half-feature chains so DVE ops still pipeline
                l1_on = tk < T
                l2_on = 0 <= sig < T
                units = []
                if l1_on:
                    rngs = [(0, SC)] if l2_on else [(0, SC // 2), (SC // 2, SC)]
                    for lo, hi in rngs:
                        units.append((u1, cms1, bt1, c1_slice, tk + 1, lo, hi, 0))
                if l2_on:
                    rngs = [(0, SC)] if l1_on else [(0, SC // 2), (SC // 2, SC)]
                    for lo, hi in rngs:
                        units.append((u2, cms2, bt2, c2_slice, sig + 1,
                                      lo, hi, SC))
                # i2
                for u, cms, bt, cfn, nt, lo, hi, off in units:
                    nc.vector.tensor_tensor(
                        out=a[:, off + lo:off + hi], in0=u[:, lo:hi],
                        in1=cms[:, lo:hi], op=Op.subtract)
                # i1 (skip layer-2's last: its output is never consumed)
                for u, cms, bt, cfn, nt, lo, hi, off in units:
                    if off == SC and nt >= T:
                        continue
                    nc.vector.scalar_tensor_tensor(
                        out=cms[:, lo:hi], in0=a[:, off + lo:off + hi], scalar=0.0,
                        in1=cfn(nt, lo, hi),
                        op0=Op.is_gt, op1=Op.subtract)
                # i3
                for u, cms, bt, cfn, nt, lo, hi, off in units:
                    nc.vector.tensor_tensor(
                        out=u[:, lo:hi], in0=a[:, off + lo:off + hi],
                        in1=bt[:, lo:hi], op=Op.mult)
                # i4 (ScalarE): spikes as Sign(a) in {-1, +1}
                sblk = s12_blocks[tk // TB]
                base = (tk % TB) * 2 * SC
                nc.scalar.activation(
                    out=sblk[:, base:base + 2 * SC],
                    in_=a, func=Sign)

            # ---- tail: last act block, transpose, output scan, DMA out ----
            # first half (act blocks 0..NB3/2-1 are long done): transpose + scan
            TH = T // 2
            nc.sync.dma_start(
                out=act64[:, :TH],
                in_=act_arr.rearrange("p (b t) -> p b t", t=T)[:, :, :TH])
            nc.vector.tensor_tensor_scan(
                out=out_sb[:, :TH], data0=decay[:, :TH], data1=act64[:, :TH],
                initial=0.0, op0=Op.mult, op1=Op.add)
            nc.sync.dma_start(out=out_e[:, :TH], in_=out_sb[:, :TH])
            g3_block(NB3 - 1)
            nc.sync.dma_start(
                out=act64[:, TH:],
                in_=act_arr.rearrange("p (b t) -> p b t", t=T)[:, :, TH:])
            nc.vector.tensor_tensor_scan(
                out=out_sb[:, TH:], data0=decay[:, TH:], data1=act64[:, TH:],
                initial=out_sb[:, TH - 1:TH], op0=Op.mult, op1=Op.add)
            nc.sync.dma_start(out=out_e[:, TH:], in_=out_sb[:, TH:])

    nc.compile()
    return nc


def _prep_inputs(inputs):
    """Host-side prep: normalized split-precision weights + per-core shards."""
    x = np.asarray(inputs["batch"], np.float32)        # [B, T, NIN]
    W1 = np.asarray(inputs["W1"], np.float32); b1 = np.asarray(inputs["b1"], np.float32)
    W2 = np.asarray(inputs["W2"], np.float32); b2 = np.asarray(inputs["b2"], np.float32)
    Wa = np.asarray(inputs["Wa"], np.float32); ba = np.asarray(inputs["ba"], np.float32)
    beta1 = np.clip(np.asarray(inputs["beta1"], np.float32), 0, 1)
    thr1 = np.asarray(inputs["thr1"], np.float32)
    beta2 = np.clip(np.asarray(inputs["beta2"], np.float32), 0, 1)
    thr2 = np.asarray(inputs["thr2"], np.float32)
    mn = float(np.float32(inputs["inp_min"])); mx = float(np.float32(inputs["inp_max"]))
    R = mx - mn

    W1n = (W1 / R) / thr1[:, None]
    b1eff = b1 - (mn / R) * W1.sum(1)
    b1n = b1eff / thr1 + beta1 - 1.0

    W2n = W2 / thr2[:, None]
    b2n = b2 / thr2 + beta2 - 1.0
    W2e = W2n / 2
    b2tot = b2n + W2n.sum(1) / 2
    W2hi, W2lo = _bf16_split(W2e)

    Wae = Wa / 2
    batot = ba + Wa.sum(1) / 2
    Wahi, Walo = _bf16_split(Wae)

    def chunked_w2(w):  # [512,512] -> W2eT chunk layout: col (j*4+m)*128 + mc
        wt = np.asarray(w).T
        outw = np.zeros((128, 16 * 128), w.dtype)
        for j in range(4):
            for m in range(4):
                outw[:, (j * 4 + m) * 128:(j * 4 + m + 1) * 128] = \
                    wt[j * 128:(j + 1) * 128, m * 128:(m + 1) * 128]
        return outw

    def chunked_wa(w):  # [4,512] -> WaeT chunks: col j*4 + a
        wt = np.asarray(w).T
        outw = np.zeros((128, 16), w.dtype)
        for j in range(4):
            outw[:, j * 4:(j + 1) * 4] = wt[j * 128:(j + 1) * 128, :]
        return outw

    def beta_tile(beta):
        return np.ascontiguousarray(
            np.repeat(beta.reshape(4, 128).T[:, :, None], BL, 2).reshape(128, SC))

    common = {
        "w2hi": np.ascontiguousarray(chunked_w2(W2hi)),
        "w2lo": np.ascontiguousarray(chunked_w2(W2lo)),
        "wahi": np.ascontiguousarray(chunked_wa(Wahi)),
        "walo": np.ascontiguousarray(chunked_wa(Walo)),
        "b2n": np.ascontiguousarray(b2tot.reshape(4, 128).T),
        "ban": np.ascontiguousarray(batot.reshape(NACT, 1)),
        "bt1": beta_tile(beta1),
        "bt2": beta_tile(beta2),
    }
    common["nbt1"] = np.ascontiguousarray(-common["bt1"])
    common["nbt2"] = np.ascontiguousarray(-common["bt2"])

    # per-core host-precomputed c1 = W1n @ x_t + b1n (exact fp32),
    # laid out [128, t*SC + j*BL + b] and split into NQ chunks of QS steps
    xt = x.transpose(1, 0, 2)  # [T, B, NIN]
    in_maps = []
    for c in range(N_CORES):
        xs = xt[:, c * BL:(c + 1) * BL, :]                    # [T, BL, NIN]
        c1 = np.einsum('hk,tbk->thb', W1n, xs).astype(np.float32) \
            + b1n[None, :, None]                              # [T, 512, BL]
        c1 = np.concatenate(
            [c1, np.broadcast_to(b1n[None, :, None], (1, H, BL))], 0)  # t=256
        c1c = np.ascontiguousarray(
            c1.reshape(T + 1, 4, 128, BL).transpose(2, 0, 1, 3)
            .reshape(128, (T + 1) * SC))
        pad = np.zeros((128, NQ * QS * SC - (T + 1) * SC), np.float32)
        c1full = np.concatenate([c1c, pad], 1)
        m = dict(common)
        for i in range(NQ):
            m[f"c1q{i}"] = np.ascontiguousarray(
                c1full[:, i * QS * SC:(i + 1) * QS * SC])
        in_maps.append(m)
    return in_maps


def _get_nc():
    if "nc" not in _cache:
        _cache["nc"] = _build_program()
    return _cache["nc"]


def _run(inputs, trace=False, trace_kwargs=None):
    nc = _get_nc()
    in_maps = _prep_inputs(inputs)
    res = run_bass_kernel_spmd(nc, in_maps, core_ids=list(range(N_CORES)),
                               trace=trace, **(trace_kwargs or {}))
    outs = []
    for c in range(N_CORES):
        o = np.asarray(res.results[c]["out"], np.float32)  # [(a,b), t]
        outs.append(o.reshape(NACT, BL, T).transpose(2, 1, 0))  # [T, BL, 4]
    full = np.concatenate(outs, axis=1)          # [T, B, 4]
    return full.reshape(1, T, B * NACT).astype(np.float32), res


def kernel(**inputs) -> np.ndarray:
    out, _ = _run(inputs, trace=False)
    return out
